# revision 11
# baseline (speedup 1.0000x reference)
"""MoE (63 routed experts top-7 + 1 shared expert) Trainium2 kernel.

Strategy: expert-parallel sparse dispatch. The router (softmax + top-k,
~0.3% of FLOPs) runs on host; tokens are gathered expert-major into
fixed-capacity weight slots, which are distributed across 8 NeuronCores.
Each core runs an identical (SPMD) Bass program in two phases:

  * routed phase: 8 slots x 1024 tokens in fp8e4m3 with DoubleRow
    matmuls (2 k-tiles per instruction, ~2x Tensor throughput).
    Weights are pre-scaled x64 and activations x16 on host to stay out
    of the e4m3 subnormal range; the product scale is divided back out
    via the activation unit's scale port. Gate-damping (sum g_i << 1)
    keeps the routed fp8 error small in the final output.
  * shared phase: 1 slot x 1024 tokens in fp16 (the shared expert's
    output is ungated so fp8 error there would dominate the result).

Per slot: a 1280->1280 Linear + exact GELU + 1280->1280 Linear,
feature-major (features on partitions, tokens on the free dim) so
weights need no transpose and biases ride the activation unit's
per-partition bias port. Outputs are gathered and gate-weighted back on
host in the reference's exact accumulation order.
"""

import math
import sys

sys.path.insert(0, "/opt/trn_rl_repo")

import numpy as np

D = 1280          # model dim
I = 1280          # expert inter dim
EXPERTS = 63      # routed experts
TOPK = 7          # routed top-k
CAP = 1024        # tokens per weight slot
CHUNK = 512       # tokens per matmul (PSUM bank limit)
KT = D // 128     # 10 contraction tiles
NCORES = 8

# fp8 scaling: weights x64 and activations x16 keep values out of the e4m3
# subnormal range (|v| < 2^-6); the product scale 1/1024 (layer 1) and 1/64
# (layer 2) is folded into the activation's scale port.
WSCALE = 64.0
XSCALE = 16.0

_PROGRAM_CACHE = {}


# ----------------------------------------------------------------- router

def _route(x2d, wr, br):
    """f32 softmax + top-k, matching jax.nn.softmax / jax.lax.top_k."""
    logits = (x2d @ wr + br).astype(np.float32)
    logits -= logits.max(-1, keepdims=True)
    np.exp(logits, out=logits)
    aff = logits / logits.sum(-1, keepdims=True)
    idx = np.argsort(-aff, axis=-1, kind="stable")[:, :TOPK]
    vals = np.take_along_axis(aff, idx, axis=-1)
    return idx.astype(np.int32), vals.astype(np.float32)


def _build_plan(T, idx):
    """Routed (token, expert) pairs packed expert-major into CAP-token
    pieces -> 8 cores x S8 fp8 slots (dummy-padded); shared tokens ->
    8 cores x S16 fp16 slots."""
    flat = idx.ravel()
    order = np.argsort(flat, kind="stable")          # expert-major slot order
    tok_of = (order // TOPK).astype(np.int64)
    counts = np.bincount(flat, minlength=EXPERTS)
    offs = np.concatenate([[0], np.cumsum(counts)])

    pieces = []  # (expert, a, b)  [a:b) into the expert-major order
    for e in range(EXPERTS):
        a, b = int(offs[e]), int(offs[e + 1])
        while a < b:
            n = min(CAP, b - a)
            pieces.append((e, a, a + n))
            a += n

    S8 = max(1, math.ceil(len(pieces) / NCORES))
    pieces += [(-1, 0, 0)] * (NCORES * S8 - len(pieces))   # dummy cells

    # shared tokens: contiguous ranges, T/NCORES per core in S16 slots
    per_core = math.ceil(T / NCORES)
    S16 = max(1, math.ceil(per_core / CAP))
    shared = []  # (a, b) token range per (core, slot)
    for c in range(NCORES):
        t0, t1 = min(c * per_core, T), min((c + 1) * per_core, T)
        for j in range(S16):
            a = min(t0 + j * CAP, t1)
            shared.append((a, min(a + CAP, t1)))
    return pieces, S8, shared, S16, order, tok_of


# ----------------------------------------------------------- device program

def _build_program(S8, S16, M):
    import concourse.mybir as mybir
    import concourse.tile as tile
    from concourse import bacc

    f32 = mybir.dt.float32
    fp8 = mybir.dt.float8e4
    fp16 = mybir.dt.float16

    M8 = S8 * CAP
    M16 = S16 * CAP
    assert M == M8 + M16

    nc = bacc.Bacc("TRN2", target_bir_lowering=False, debug=False,
                   enable_asserts=False, num_devices=NCORES)
    xT8 = nc.dram_tensor("xT8", [KT, 128, M8], fp8, kind="ExternalInput").ap()
    xT16 = nc.dram_tensor("xT16", [KT, 128, M16], fp16, kind="ExternalInput").ap()
    w1s8 = nc.dram_tensor("w1s8", [S8, KT, 128, KT, 128], fp8, kind="ExternalInput").ap()
    w2s8 = nc.dram_tensor("w2s8", [S8, KT, 128, KT, 128], fp8, kind="ExternalInput").ap()
    w1s16 = nc.dram_tensor("w1s16", [S16, KT, 128, KT, 128], fp16, kind="ExternalInput").ap()
    w2s16 = nc.dram_tensor("w2s16", [S16, KT, 128, KT, 128], fp16, kind="ExternalInput").ap()
    b1s = nc.dram_tensor("b1s", [S8 + S16, 128, KT], f32, kind="ExternalInput").ap()
    b2s = nc.dram_tensor("b2s", [S8 + S16, 128, KT], f32, kind="ExternalInput").ap()
    yT = nc.dram_tensor("yT", [KT, 128, M], f32, kind="ExternalOutput").ap()

    CPS = CAP // CHUNK  # chunks per slot
    Gelu = mybir.ActivationFunctionType.Gelu
    Ident = mybir.ActivationFunctionType.Identity
    DR = mybir.MatmulPerfMode.DoubleRow

    with tile.TileContext(nc) as tc:
        with (
            tc.tile_pool(name="xa", bufs=3) as xa,
            tc.tile_pool(name="w1p", bufs=4) as w1p,
            tc.tile_pool(name="w2p", bufs=4) as w2p,
            tc.tile_pool(name="hp", bufs=3) as hp,
            tc.tile_pool(name="yo", bufs=6) as yo,
            tc.tile_pool(name="bp", bufs=2) as bp,
            tc.tile_pool(name="ps", bufs=8, space="PSUM") as ps,
        ):
            def slot(s, xcol0, ycol0, xsrc, w1src, w2src, widx, in_dt, kstep,
                     s1, s2):
                pm = DR if kstep == 2 else None
                b1t = bp.tile([128, KT], f32, tag="b1", name="b1t")
                nc.sync.dma_start(out=b1t[:, :], in_=b1s[s])
                b2t = bp.tile([128, KT], f32, tag="b2", name="b2t")
                nc.sync.dma_start(out=b2t[:, :], in_=b2s[s])

                xc = []
                for c in range(CPS):
                    xt = xa.tile([128, KT, CHUNK], in_dt, tag=f"x{kstep}",
                                 name="xt")
                    for k in range(KT):
                        # SWDGE: keeps HWDGE free for the slot's weight loads
                        nc.gpsimd.dma_start(
                            out=xt[:, k, :],
                            in_=xsrc[k, :, xcol0 + c * CHUNK:
                                     xcol0 + (c + 1) * CHUNK])
                    xc.append(xt)

                hc = [hp.tile([128, KT, CHUNK], in_dt, tag=f"h{kstep}",
                              name=f"h{c}") for c in range(CPS)]

                # layer 1: h = gelu((x @ w1) * s1 + b1), feature-major
                for io in range(KT):
                    w1t = w1p.tile([128, KT, 128], in_dt, tag=f"w1{kstep}",
                                   name="w1t")
                    nc.sync.dma_start(out=w1t[:, :, :], in_=w1src[widx, io])
                    pts = [ps.tile([128, CHUNK], f32, tag="ps", name="pt")
                           for _ in range(CPS)]
                    # k outer, c inner: one weight (pair) load covers CPS
                    # matmuls so LDWEIGHTS hides under the streaming
                    for k in range(0, KT, kstep):
                        for c in range(CPS):
                            nc.tensor.matmul(pts[c][:, :],
                                             w1t[:, k:k + kstep, :],
                                             xc[c][:, k:k + kstep, :],
                                             start=(k == 0),
                                             stop=(k + kstep >= KT),
                                             perf_mode=pm)
                    for c in range(CPS):
                        nc.scalar.activation(hc[c][:, io, :], pts[c][:, :],
                                             Gelu, bias=b1t[:, io:io + 1],
                                             scale=s1)

                # layer 2: y = (h @ w2) * s2 + b2
                for io in range(KT):
                    w2t = w2p.tile([128, KT, 128], in_dt, tag=f"w2{kstep}",
                                   name="w2t")
                    nc.sync.dma_start(out=w2t[:, :, :], in_=w2src[widx, io])
                    pts = [ps.tile([128, CHUNK], f32, tag="ps", name="pt")
                           for _ in range(CPS)]
                    for k in range(0, KT, kstep):
                        for c in range(CPS):
                            nc.tensor.matmul(pts[c][:, :],
                                             w2t[:, k:k + kstep, :],
                                             hc[c][:, k:k + kstep, :],
                                             start=(k == 0),
                                             stop=(k + kstep >= KT),
                                             perf_mode=pm)
                    for c in range(CPS):
                        yt = yo.tile([128, CHUNK], f32, tag="y", name="yt")
                        nc.scalar.activation(yt[:, :], pts[c][:, :], Ident,
                                             bias=b2t[:, io:io + 1], scale=s2)
                        nc.sync.dma_start(
                            out=yT[io, :, ycol0 + c * CHUNK:
                                   ycol0 + (c + 1) * CHUNK],
                            in_=yt[:, :])

            for s in range(S8):
                slot(s, s * CAP, s * CAP, xT8, w1s8, w2s8, s,
                     mybir.dt.float8e4, 2,
                     1.0 / (WSCALE * XSCALE), 1.0 / WSCALE)
            for s in range(S16):
                slot(S8 + s, s * CAP, M8 + s * CAP, xT16, w1s16, w2s16, s,
                     fp16, 1, 1.0, 1.0)
    nc.compile()
    return nc


def _get_program(S8, S16, M):
    key = (S8, S16, M)
    if key not in _PROGRAM_CACHE:
        _PROGRAM_CACHE[key] = _build_program(S8, S16, M)
    return _PROGRAM_CACHE[key]


# ------------------------------------------------------------------ kernel

def _arrange_w(w):
    """[D, I] -> [io, p, ko, c] so each (slot, io) block DMAs contiguously
    into an SBUF tile laid out [partition, ko, col]."""
    return np.ascontiguousarray(
        w.reshape(KT, 128, KT, 128).transpose(2, 1, 0, 3))


def _q8(a, scale, dt):
    """Scale + saturate to TRN e4m3 range (+-240) before fp8 downcast."""
    return np.clip(a * np.float32(scale), -240.0, 240.0).astype(dt)


def kernel(x, sw1, sb1, sw2, sb2, rw1, rb1, rw2, rb2, wr, br, _trace=False):
    import ml_dtypes
    from concourse.bass_utils import run_bass_kernel_spmd

    fp8 = ml_dtypes.float8_e4m3

    x = np.asarray(x, dtype=np.float32)
    B, Sq, _ = x.shape
    T = B * Sq
    xf = np.ascontiguousarray(x.reshape(T, D))

    idx, vals = _route(xf, np.asarray(wr, np.float32), np.asarray(br, np.float32))
    pieces, S8, shared, S16, order, tok_of = _build_plan(T, idx)
    M8, M16 = S8 * CAP, S16 * CAP
    M = M8 + M16

    rw1 = np.asarray(rw1, np.float32); rw2 = np.asarray(rw2, np.float32)
    rb1 = np.asarray(rb1, np.float32); rb2 = np.asarray(rb2, np.float32)
    sw1 = np.asarray(sw1, np.float32); sw2 = np.asarray(sw2, np.float32)
    sb1 = np.asarray(sb1, np.float32); sb2 = np.asarray(sb2, np.float32)

    w1a = [_q8(_arrange_w(rw1[e]), WSCALE, fp8) for e in range(EXPERTS)]
    w2a = [_q8(_arrange_w(rw2[e]), WSCALE, fp8) for e in range(EXPERTS)]
    sw1a = _arrange_w(sw1).astype(np.float16)
    sw2a = _arrange_w(sw2).astype(np.float16)
    b1a = [np.ascontiguousarray(rb1[e].reshape(KT, 128).T) for e in range(EXPERTS)]
    b2a = [np.ascontiguousarray(rb2[e].reshape(KT, 128).T) for e in range(EXPERTS)]
    sb1a = np.ascontiguousarray(sb1.reshape(KT, 128).T)
    sb2a = np.ascontiguousarray(sb2.reshape(KT, 128).T)

    xfT = np.ascontiguousarray(xf.T)                 # [D, T] f32
    xfT8 = _q8(xfT, XSCALE, fp8)                     # routed operand
    xfT16 = xfT.astype(np.float16)                   # shared operand

    in_maps = []
    for core in range(NCORES):
        xT8c = np.zeros((D, M8), dtype=fp8)
        xT16c = np.zeros((D, M16), dtype=np.float16)
        w1c8 = np.zeros((S8, KT, 128, KT, 128), dtype=fp8)
        w2c8 = np.zeros((S8, KT, 128, KT, 128), dtype=fp8)
        w1c16 = np.zeros((S16, KT, 128, KT, 128), dtype=np.float16)
        w2c16 = np.zeros((S16, KT, 128, KT, 128), dtype=np.float16)
        b1c = np.zeros((S8 + S16, 128, KT), dtype=np.float32)
        b2c = np.zeros((S8 + S16, 128, KT), dtype=np.float32)
        for j in range(S8):
            e, a, b = pieces[core * S8 + j]
            if e >= 0:
                xT8c[:, j * CAP: j * CAP + (b - a)] = xfT8[:, tok_of[a:b]]
                w1c8[j] = w1a[e]; w2c8[j] = w2a[e]
                b1c[j] = b1a[e]; b2c[j] = b2a[e]
        for j in range(S16):
            a, b = shared[core * S16 + j]
            xT16c[:, j * CAP: j * CAP + (b - a)] = xfT16[:, a:b]
            w1c16[j] = sw1a; w2c16[j] = sw2a
            b1c[S8 + j] = sb1a; b2c[S8 + j] = sb2a
        in_maps.append({
            "xT8": xT8c.reshape(KT, 128, M8),
            "xT16": xT16c.reshape(KT, 128, M16),
            "w1s8": w1c8, "w2s8": w2c8,
            "w1s16": w1c16, "w2s16": w2c16,
            "b1s": b1c, "b2s": b2c,
        })

    nc = _get_program(S8, S16, M)
    res = run_bass_kernel_spmd(nc, in_maps, core_ids=list(range(NCORES)),
                               trace=_trace)
    kernel.last_result = res

    TK = T * TOPK
    gated = np.empty((TK, D), dtype=np.float32)   # expert-major rows
    shared_out = np.empty((T, D), dtype=np.float32)
    for core in range(NCORES):
        Y = res.results[core]["yT"].reshape(D, M)
        for j in range(S8):
            e, a, b = pieces[core * S8 + j]
            if e >= 0:
                gated[a:b] = Y[:, j * CAP: j * CAP + (b - a)].T
        for j in range(S16):
            a, b = shared[core * S16 + j]
            shared_out[a:b] = Y[:, M8 + j * CAP: M8 + j * CAP + (b - a)].T

    g = vals.ravel()[order].astype(np.float32)
    gated *= g[:, None]
    ord2 = np.argsort(tok_of, kind="stable")      # token-major, expert asc
    routed = gated[ord2].reshape(T, TOPK, D).sum(axis=1, dtype=np.float32)

    out = shared_out + routed + xf
    return out.reshape(B, Sq, D).astype(np.float32)


kernel.last_result = None


# revision 14
# speedup vs baseline: 1.0115x; 1.0115x over previous
"""MoE (63 routed experts top-7 + 1 shared expert) Trainium2 kernel.

Strategy: expert-parallel sparse dispatch. The router (softmax + top-k,
~0.3% of FLOPs) runs on host; tokens are gathered expert-major into
fixed-capacity weight slots, which are distributed across 8 NeuronCores.
Each core runs an identical (SPMD) Bass program in two phases:

  * routed phase: 8 slots x 1024 tokens in fp8e4m3 with DoubleRow
    matmuls (2 k-tiles per instruction, ~2x Tensor throughput).
    Weights are pre-scaled x64 and activations x16 on host to stay out
    of the e4m3 subnormal range; the product scale is divided back out
    via the activation unit's scale port. Gate-damping (sum g_i << 1)
    keeps the routed fp8 error small in the final output.
  * shared phase: 1 slot x 1024 tokens in fp16 (the shared expert's
    output is ungated so fp8 error there would dominate the result).

Per slot: a 1280->1280 Linear + exact GELU + 1280->1280 Linear,
feature-major (features on partitions, tokens on the free dim) so
weights need no transpose and biases ride the activation unit's
per-partition bias port. Outputs are gathered and gate-weighted back on
host in the reference's exact accumulation order.
"""

import math
import sys

sys.path.insert(0, "/opt/trn_rl_repo")

import numpy as np

D = 1280          # model dim
I = 1280          # expert inter dim
EXPERTS = 63      # routed experts
TOPK = 7          # routed top-k
CAP = 1024        # tokens per weight slot
CHUNK = 512       # tokens per matmul (PSUM bank limit)
KT = D // 128     # 10 contraction tiles
NCORES = 8

# fp8 scaling: weights x64 and activations x16 keep values out of the e4m3
# subnormal range (|v| < 2^-6); the product scale 1/1024 (layer 1) and 1/64
# (layer 2) is folded into the activation's scale port.
WSCALE = 64.0
XSCALE = 16.0

_PROGRAM_CACHE = {}


# ----------------------------------------------------------------- router

def _route(x2d, wr, br):
    """f32 softmax + top-k, matching jax.nn.softmax / jax.lax.top_k."""
    logits = (x2d @ wr + br).astype(np.float32)
    logits -= logits.max(-1, keepdims=True)
    np.exp(logits, out=logits)
    aff = logits / logits.sum(-1, keepdims=True)
    idx = np.argsort(-aff, axis=-1, kind="stable")[:, :TOPK]
    vals = np.take_along_axis(aff, idx, axis=-1)
    return idx.astype(np.int32), vals.astype(np.float32)


def _build_plan(T, idx):
    """Routed (token, expert) pairs packed expert-major into CAP-token
    pieces -> 8 cores x S8 fp8 slots (dummy-padded); shared tokens ->
    8 cores x S16 fp16 slots."""
    flat = idx.ravel()
    order = np.argsort(flat, kind="stable")          # expert-major slot order
    tok_of = (order // TOPK).astype(np.int64)
    counts = np.bincount(flat, minlength=EXPERTS)
    offs = np.concatenate([[0], np.cumsum(counts)])

    pieces = []  # (expert, a, b)  [a:b) into the expert-major order
    for e in range(EXPERTS):
        a, b = int(offs[e]), int(offs[e + 1])
        while a < b:
            n = min(CAP, b - a)
            pieces.append((e, a, a + n))
            a += n

    S8 = max(1, math.ceil(len(pieces) / NCORES))
    pieces += [(-1, 0, 0)] * (NCORES * S8 - len(pieces))   # dummy cells

    # shared tokens: contiguous ranges, T/NCORES per core in S16 slots
    per_core = math.ceil(T / NCORES)
    S16 = max(1, math.ceil(per_core / CAP))
    shared = []  # (a, b) token range per (core, slot)
    for c in range(NCORES):
        t0, t1 = min(c * per_core, T), min((c + 1) * per_core, T)
        for j in range(S16):
            a = min(t0 + j * CAP, t1)
            shared.append((a, min(a + CAP, t1)))
    return pieces, S8, shared, S16, order, tok_of


# ----------------------------------------------------------- device program

def _build_program(S8, S16, M):
    import concourse.mybir as mybir
    import concourse.tile as tile
    from concourse import bacc

    f32 = mybir.dt.float32
    fp8 = mybir.dt.float8e4
    fp16 = mybir.dt.float16

    M8 = S8 * CAP
    M16 = S16 * CAP
    assert M == M8 + M16

    nc = bacc.Bacc("TRN2", target_bir_lowering=False, debug=False,
                   enable_asserts=False, num_devices=NCORES)
    xT8 = nc.dram_tensor("xT8", [KT, 128, M8], fp8, kind="ExternalInput").ap()
    xT16 = nc.dram_tensor("xT16", [KT, 128, M16], fp16, kind="ExternalInput").ap()
    w1s8 = nc.dram_tensor("w1s8", [S8, KT, 128, KT, 128], fp8, kind="ExternalInput").ap()
    w2s8 = nc.dram_tensor("w2s8", [S8, KT, 128, KT, 128], fp8, kind="ExternalInput").ap()
    w1s16 = nc.dram_tensor("w1s16", [S16, KT, 128, KT, 128], fp16, kind="ExternalInput").ap()
    w2s16 = nc.dram_tensor("w2s16", [S16, KT, 128, KT, 128], fp16, kind="ExternalInput").ap()
    b1s = nc.dram_tensor("b1s", [S8 + S16, 128, KT], f32, kind="ExternalInput").ap()
    b2s = nc.dram_tensor("b2s", [S8 + S16, 128, KT], f32, kind="ExternalInput").ap()
    yT = nc.dram_tensor("yT", [KT, 128, M], f32, kind="ExternalOutput").ap()

    CPS = CAP // CHUNK  # chunks per slot
    Gelu = mybir.ActivationFunctionType.Gelu
    DR = mybir.MatmulPerfMode.DoubleRow

    with tile.TileContext(nc) as tc:
        with (
            tc.tile_pool(name="xa", bufs=3) as xa,
            tc.tile_pool(name="w1p", bufs=4) as w1p,
            tc.tile_pool(name="w2p", bufs=4) as w2p,
            tc.tile_pool(name="hp", bufs=3) as hp,
            tc.tile_pool(name="yo", bufs=6) as yo,
            tc.tile_pool(name="bp", bufs=4) as bp,
            tc.tile_pool(name="ps", bufs=8, space="PSUM") as ps,
        ):
            def slot(s, xcol0, ycol0, xsrc, w1src, w2src, widx, in_dt, kstep,
                     s1, s2):
                pm = DR if kstep == 2 else None
                b1t = bp.tile([128, KT], f32, tag="b1", name="b1t")
                nc.sync.dma_start(out=b1t[:, :], in_=b1s[s])
                b2t = bp.tile([128, KT], f32, tag="b2", name="b2t")
                nc.sync.dma_start(out=b2t[:, :], in_=b2s[s])

                xc = []
                for c in range(CPS):
                    xt = xa.tile([128, KT, CHUNK], in_dt, tag=f"x{kstep}",
                                 name="xt")
                    for k in range(KT):
                        # SWDGE: keeps HWDGE free for the slot's weight loads
                        nc.gpsimd.dma_start(
                            out=xt[:, k, :],
                            in_=xsrc[k, :, xcol0 + c * CHUNK:
                                     xcol0 + (c + 1) * CHUNK])
                    xc.append(xt)

                hc = [hp.tile([128, KT, CHUNK], in_dt, tag=f"h{kstep}",
                              name=f"h{c}") for c in range(CPS)]

                # layer 1: h = gelu((x @ w1) * s1 + b1), feature-major
                for io in range(KT):
                    w1t = w1p.tile([128, KT, 128], in_dt, tag=f"w1{kstep}",
                                   name="w1t")
                    nc.sync.dma_start(out=w1t[:, :, :], in_=w1src[widx, io])
                    pts = [ps.tile([128, CHUNK], f32, tag="ps", name="pt")
                           for _ in range(CPS)]
                    # k outer, c inner: one weight (pair) load covers CPS
                    # matmuls so LDWEIGHTS hides under the streaming
                    for k in range(0, KT, kstep):
                        for c in range(CPS):
                            nc.tensor.matmul(pts[c][:, :],
                                             w1t[:, k:k + kstep, :],
                                             xc[c][:, k:k + kstep, :],
                                             start=(k == 0),
                                             stop=(k + kstep >= KT),
                                             perf_mode=pm)
                    for c in range(CPS):
                        nc.scalar.activation(hc[c][:, io, :], pts[c][:, :],
                                             Gelu, bias=b1t[:, io:io + 1],
                                             scale=s1)

                # layer 2: y = (h @ w2) * s2 + b2
                for io in range(KT):
                    w2t = w2p.tile([128, KT, 128], in_dt, tag=f"w2{kstep}",
                                   name="w2t")
                    nc.sync.dma_start(out=w2t[:, :, :], in_=w2src[widx, io])
                    pts = [ps.tile([128, CHUNK], f32, tag="ps", name="pt")
                           for _ in range(CPS)]
                    for k in range(0, KT, kstep):
                        for c in range(CPS):
                            nc.tensor.matmul(pts[c][:, :],
                                             w2t[:, k:k + kstep, :],
                                             hc[c][:, k:k + kstep, :],
                                             start=(k == 0),
                                             stop=(k + kstep >= KT),
                                             perf_mode=pm)
                    for c in range(CPS):
                        yt = yo.tile([128, CHUNK], f32, tag="y", name="yt")
                        # drain layer-2 PSUM on the (otherwise idle) Vector
                        # engine: y = psum * s2 + b2.  DVE at ~267ns/tile
                        # beats PE production (~1080ns/tile), so slot tails
                        # don't serialize behind the Scalar engine.
                        nc.vector.tensor_scalar(
                            yt[:, :], pts[c][:, :], s2, b2t[:, io:io + 1],
                            mybir.AluOpType.mult, mybir.AluOpType.add)
                        nc.sync.dma_start(
                            out=yT[io, :, ycol0 + c * CHUNK:
                                   ycol0 + (c + 1) * CHUNK],
                            in_=yt[:, :])

            for s in range(S8):
                slot(s, s * CAP, s * CAP, xT8, w1s8, w2s8, s,
                     mybir.dt.float8e4, 2,
                     1.0 / (WSCALE * XSCALE), 1.0 / WSCALE)
            for s in range(S16):
                slot(S8 + s, s * CAP, M8 + s * CAP, xT16, w1s16, w2s16, s,
                     fp16, 1, 1.0, 1.0)
    nc.compile()
    return nc


def _get_program(S8, S16, M):
    key = (S8, S16, M)
    if key not in _PROGRAM_CACHE:
        _PROGRAM_CACHE[key] = _build_program(S8, S16, M)
    return _PROGRAM_CACHE[key]


# ------------------------------------------------------------------ kernel

def _arrange_w(w):
    """[D, I] -> [io, p, ko, c] so each (slot, io) block DMAs contiguously
    into an SBUF tile laid out [partition, ko, col]."""
    return np.ascontiguousarray(
        w.reshape(KT, 128, KT, 128).transpose(2, 1, 0, 3))


def _q8(a, scale, dt):
    """Scale + saturate to TRN e4m3 range (+-240) before fp8 downcast."""
    return np.clip(a * np.float32(scale), -240.0, 240.0).astype(dt)


def kernel(x, sw1, sb1, sw2, sb2, rw1, rb1, rw2, rb2, wr, br, _trace=False):
    import ml_dtypes
    from concourse.bass_utils import run_bass_kernel_spmd

    fp8 = ml_dtypes.float8_e4m3

    x = np.asarray(x, dtype=np.float32)
    B, Sq, _ = x.shape
    T = B * Sq
    xf = np.ascontiguousarray(x.reshape(T, D))

    idx, vals = _route(xf, np.asarray(wr, np.float32), np.asarray(br, np.float32))
    pieces, S8, shared, S16, order, tok_of = _build_plan(T, idx)
    M8, M16 = S8 * CAP, S16 * CAP
    M = M8 + M16

    rw1 = np.asarray(rw1, np.float32); rw2 = np.asarray(rw2, np.float32)
    rb1 = np.asarray(rb1, np.float32); rb2 = np.asarray(rb2, np.float32)
    sw1 = np.asarray(sw1, np.float32); sw2 = np.asarray(sw2, np.float32)
    sb1 = np.asarray(sb1, np.float32); sb2 = np.asarray(sb2, np.float32)

    w1a = [_q8(_arrange_w(rw1[e]), WSCALE, fp8) for e in range(EXPERTS)]
    w2a = [_q8(_arrange_w(rw2[e]), WSCALE, fp8) for e in range(EXPERTS)]
    sw1a = _arrange_w(sw1).astype(np.float16)
    sw2a = _arrange_w(sw2).astype(np.float16)
    b1a = [np.ascontiguousarray(rb1[e].reshape(KT, 128).T) for e in range(EXPERTS)]
    b2a = [np.ascontiguousarray(rb2[e].reshape(KT, 128).T) for e in range(EXPERTS)]
    sb1a = np.ascontiguousarray(sb1.reshape(KT, 128).T)
    sb2a = np.ascontiguousarray(sb2.reshape(KT, 128).T)

    xfT = np.ascontiguousarray(xf.T)                 # [D, T] f32
    xfT8 = _q8(xfT, XSCALE, fp8)                     # routed operand
    xfT16 = xfT.astype(np.float16)                   # shared operand

    in_maps = []
    for core in range(NCORES):
        xT8c = np.zeros((D, M8), dtype=fp8)
        xT16c = np.zeros((D, M16), dtype=np.float16)
        w1c8 = np.zeros((S8, KT, 128, KT, 128), dtype=fp8)
        w2c8 = np.zeros((S8, KT, 128, KT, 128), dtype=fp8)
        w1c16 = np.zeros((S16, KT, 128, KT, 128), dtype=np.float16)
        w2c16 = np.zeros((S16, KT, 128, KT, 128), dtype=np.float16)
        b1c = np.zeros((S8 + S16, 128, KT), dtype=np.float32)
        b2c = np.zeros((S8 + S16, 128, KT), dtype=np.float32)
        for j in range(S8):
            e, a, b = pieces[core * S8 + j]
            if e >= 0:
                xT8c[:, j * CAP: j * CAP + (b - a)] = xfT8[:, tok_of[a:b]]
                w1c8[j] = w1a[e]; w2c8[j] = w2a[e]
                b1c[j] = b1a[e]; b2c[j] = b2a[e]
        for j in range(S16):
            a, b = shared[core * S16 + j]
            xT16c[:, j * CAP: j * CAP + (b - a)] = xfT16[:, a:b]
            w1c16[j] = sw1a; w2c16[j] = sw2a
            b1c[S8 + j] = sb1a; b2c[S8 + j] = sb2a
        in_maps.append({
            "xT8": xT8c.reshape(KT, 128, M8),
            "xT16": xT16c.reshape(KT, 128, M16),
            "w1s8": w1c8, "w2s8": w2c8,
            "w1s16": w1c16, "w2s16": w2c16,
            "b1s": b1c, "b2s": b2c,
        })

    nc = _get_program(S8, S16, M)
    res = run_bass_kernel_spmd(nc, in_maps, core_ids=list(range(NCORES)),
                               trace=_trace)
    kernel.last_result = res

    TK = T * TOPK
    gated = np.empty((TK, D), dtype=np.float32)   # expert-major rows
    shared_out = np.empty((T, D), dtype=np.float32)
    for core in range(NCORES):
        Y = res.results[core]["yT"].reshape(D, M)
        for j in range(S8):
            e, a, b = pieces[core * S8 + j]
            if e >= 0:
                gated[a:b] = Y[:, j * CAP: j * CAP + (b - a)].T
        for j in range(S16):
            a, b = shared[core * S16 + j]
            shared_out[a:b] = Y[:, M8 + j * CAP: M8 + j * CAP + (b - a)].T

    g = vals.ravel()[order].astype(np.float32)
    gated *= g[:, None]
    ord2 = np.argsort(tok_of, kind="stable")      # token-major, expert asc
    routed = gated[ord2].reshape(T, TOPK, D).sum(axis=1, dtype=np.float32)

    out = shared_out + routed + xf
    return out.reshape(B, Sq, D).astype(np.float32)


kernel.last_result = None


# revision 19
# speedup vs baseline: 1.0727x; 1.0605x over previous
"""MoE (63 routed experts top-7 + 1 shared expert) Trainium2 kernel.

Strategy: expert-parallel sparse dispatch. The router (softmax + top-k,
~0.3% of FLOPs) runs on host; tokens are gathered expert-major into
fixed-capacity weight slots, which are distributed across 8 NeuronCores.
Each core runs an identical (SPMD) Bass program in two phases:

  * routed phase: 8 slots x 1024 tokens in fp8e4m3 with DoubleRow
    matmuls (2 k-tiles per instruction, ~2x Tensor throughput).
    Weights are pre-scaled x64 and activations x16 on host to stay out
    of the e4m3 subnormal range; the product scale is divided back out
    via the activation unit's scale port. Gate-damping (sum g_i << 1)
    keeps the routed fp8 error small in the final output.
  * shared phase: 1 slot x 1024 tokens in fp16 (the shared expert's
    output is ungated so fp8 error there would dominate the result).

Per slot: a 1280->1280 Linear + exact GELU + 1280->1280 Linear,
feature-major (features on partitions, tokens on the free dim) so
weights need no transpose and biases ride the activation unit's
per-partition bias port. Outputs are gathered and gate-weighted back on
host in the reference's exact accumulation order.
"""

import math
import sys

sys.path.insert(0, "/opt/trn_rl_repo")

import numpy as np

D = 1280          # model dim
I = 1280          # expert inter dim
EXPERTS = 63      # routed experts
TOPK = 7          # routed top-k
CAP = 1024        # tokens per weight slot
CHUNK = 512       # tokens per matmul (PSUM bank limit)
KT = D // 128     # 10 contraction tiles
NCORES = 8

# fp8 scaling: weights x64 and activations x16 keep values out of the e4m3
# subnormal range (|v| < 2^-6); the product scale 1/1024 (layer 1) and 1/64
# (layer 2) is folded into the activation's scale port.
WSCALE = 64.0
XSCALE = 16.0

_PROGRAM_CACHE = {}


# ----------------------------------------------------------------- router

def _route(x2d, wr, br):
    """f32 softmax + top-k, matching jax.nn.softmax / jax.lax.top_k."""
    logits = (x2d @ wr + br).astype(np.float32)
    logits -= logits.max(-1, keepdims=True)
    np.exp(logits, out=logits)
    aff = logits / logits.sum(-1, keepdims=True)
    idx = np.argsort(-aff, axis=-1, kind="stable")[:, :TOPK]
    vals = np.take_along_axis(aff, idx, axis=-1)
    return idx.astype(np.int32), vals.astype(np.float32)


def _build_plan(T, idx):
    """Routed (token, expert) pairs packed expert-major into CAP-token
    pieces -> 8 cores x S8 fp8 slots (dummy-padded); shared tokens ->
    8 cores x S16 fp16 slots."""
    flat = idx.ravel()
    order = np.argsort(flat, kind="stable")          # expert-major slot order
    tok_of = (order // TOPK).astype(np.int64)
    counts = np.bincount(flat, minlength=EXPERTS)
    offs = np.concatenate([[0], np.cumsum(counts)])

    pieces = []  # (expert, a, b)  [a:b) into the expert-major order
    for e in range(EXPERTS):
        a, b = int(offs[e]), int(offs[e + 1])
        while a < b:
            n = min(CAP, b - a)
            pieces.append((e, a, a + n))
            a += n

    S8 = max(1, math.ceil(len(pieces) / NCORES))
    pieces += [(-1, 0, 0)] * (NCORES * S8 - len(pieces))   # dummy cells

    # shared tokens: contiguous ranges, T/NCORES per core in S16 slots
    per_core = math.ceil(T / NCORES)
    S16 = max(1, math.ceil(per_core / CAP))
    shared = []  # (a, b) token range per (core, slot)
    for c in range(NCORES):
        t0, t1 = min(c * per_core, T), min((c + 1) * per_core, T)
        for j in range(S16):
            a = min(t0 + j * CAP, t1)
            shared.append((a, min(a + CAP, t1)))
    return pieces, S8, shared, S16, order, tok_of


# ----------------------------------------------------------- device program

def _build_program(S8, S16, M):
    import concourse.mybir as mybir
    import concourse.tile as tile
    from concourse import bacc

    f32 = mybir.dt.float32
    fp8 = mybir.dt.float8e4
    fp16 = mybir.dt.float16

    M8 = S8 * CAP
    M16 = S16 * CAP
    assert M == M8 + M16

    nc = bacc.Bacc("TRN2", target_bir_lowering=False, debug=False,
                   enable_asserts=False, num_devices=NCORES)
    xT8 = nc.dram_tensor("xT8", [KT, 128, M8], fp8, kind="ExternalInput").ap()
    xT16 = nc.dram_tensor("xT16", [KT, 128, M16], fp16, kind="ExternalInput").ap()
    w1s8 = nc.dram_tensor("w1s8", [S8, KT, 128, KT, 128], fp8, kind="ExternalInput").ap()
    w2s8 = nc.dram_tensor("w2s8", [S8, KT, 128, KT, 128], fp8, kind="ExternalInput").ap()
    w1s16 = nc.dram_tensor("w1s16", [S16, KT, 128, KT, 128], fp16, kind="ExternalInput").ap()
    w2s16 = nc.dram_tensor("w2s16", [S16, KT, 128, KT, 128], fp16, kind="ExternalInput").ap()
    b1s = nc.dram_tensor("b1s", [S8 + S16, 128, KT], f32, kind="ExternalInput").ap()
    b2s = nc.dram_tensor("b2s", [S8 + S16, 128, KT], f32, kind="ExternalInput").ap()
    yT = nc.dram_tensor("yT", [KT, 128, M], f32, kind="ExternalOutput").ap()

    CPS = CAP // CHUNK  # chunks per slot
    Gelu = mybir.ActivationFunctionType.Gelu
    DR = mybir.MatmulPerfMode.DoubleRow

    with tile.TileContext(nc) as tc:
        with (
            tc.tile_pool(name="xa", bufs=4) as xa,
            tc.tile_pool(name="w1p", bufs=4) as w1p,
            tc.tile_pool(name="w2p", bufs=4) as w2p,
            tc.tile_pool(name="hp", bufs=3) as hp,
            tc.tile_pool(name="yo", bufs=6) as yo,
            tc.tile_pool(name="bp", bufs=4) as bp,
            tc.tile_pool(name="ps", bufs=8, space="PSUM") as ps,
        ):
            def slot(s, xcol0, ycol0, xsrc, w1src, w2src, widx, in_dt, kstep,
                     s1, s2, first=False):
                pm = DR if kstep == 2 else None
                b1t = bp.tile([128, KT], f32, tag="b1", name="b1t")
                nc.sync.dma_start(out=b1t[:, :], in_=b1s[s])
                b2t = bp.tile([128, KT], f32, tag="b2", name="b2t")
                nc.sync.dma_start(out=b2t[:, :], in_=b2s[s])

                xc = []
                for c in range(CPS):
                    xt = xa.tile([128, KT, CHUNK], in_dt, tag=f"x{kstep}",
                                 name="xt")
                    for k in range(KT):
                        # SWDGE: keeps HWDGE free for the slot's weight
                        # loads.  First slot: split across the still-idle
                        # HWDGE queues to shorten kernel startup.
                        eng = nc.gpsimd if not first or k % 2 else nc.sync
                        eng.dma_start(
                            out=xt[:, k, :],
                            in_=xsrc[k, :, xcol0 + c * CHUNK:
                                     xcol0 + (c + 1) * CHUNK])
                    xc.append(xt)

                hc = [hp.tile([128, KT, CHUNK], in_dt, tag=f"h{kstep}",
                              name=f"h{c}") for c in range(CPS)]

                # layer 1: h = gelu((x @ w1) * s1 + b1), feature-major
                for io in range(KT):
                    w1t = w1p.tile([128, KT, 128], in_dt, tag=f"w1{kstep}",
                                   name="w1t")
                    nc.sync.dma_start(out=w1t[:, :, :], in_=w1src[widx, io])
                    pts = [ps.tile([128, CHUNK], f32, tag="ps", name="pt")
                           for _ in range(CPS)]
                    # k outer, c inner: one weight (pair) load covers CPS
                    # matmuls so LDWEIGHTS hides under the streaming
                    for k in range(0, KT, kstep):
                        for c in range(CPS):
                            nc.tensor.matmul(pts[c][:, :],
                                             w1t[:, k:k + kstep, :],
                                             xc[c][:, k:k + kstep, :],
                                             start=(k == 0),
                                             stop=(k + kstep >= KT),
                                             perf_mode=pm)
                    for c in range(CPS):
                        nc.scalar.activation(hc[c][:, io, :], pts[c][:, :],
                                             Gelu, bias=b1t[:, io:io + 1],
                                             scale=s1)

                # layer 2: y = (h @ w2) * s2 + b2
                for io in range(KT):
                    w2t = w2p.tile([128, KT, 128], in_dt, tag=f"w2{kstep}",
                                   name="w2t")
                    nc.sync.dma_start(out=w2t[:, :, :], in_=w2src[widx, io])
                    pts = [ps.tile([128, CHUNK], f32, tag="ps", name="pt")
                           for _ in range(CPS)]
                    for k in range(0, KT, kstep):
                        for c in range(CPS):
                            nc.tensor.matmul(pts[c][:, :],
                                             w2t[:, k:k + kstep, :],
                                             hc[c][:, k:k + kstep, :],
                                             start=(k == 0),
                                             stop=(k + kstep >= KT),
                                             perf_mode=pm)
                    for c in range(CPS):
                        yt = yo.tile([128, CHUNK], f32, tag="y", name="yt")
                        # drain layer-2 PSUM on the (otherwise idle) Vector
                        # engine: y = psum * s2 + b2.  DVE at ~267ns/tile
                        # beats PE production (~1080ns/tile), so slot tails
                        # don't serialize behind the Scalar engine.
                        nc.vector.tensor_scalar(
                            yt[:, :], pts[c][:, :], s2, b2t[:, io:io + 1],
                            mybir.AluOpType.mult, mybir.AluOpType.add)
                        # y-stores ride the Scalar HWDGE queue so the Sync
                        # queue can issue the NEXT slot's weight loads
                        # without queuing behind 20 gated stores
                        nc.scalar.dma_start(
                            out=yT[io, :, ycol0 + c * CHUNK:
                                   ycol0 + (c + 1) * CHUNK],
                            in_=yt[:, :])

            for s in range(S8):
                slot(s, s * CAP, s * CAP, xT8, w1s8, w2s8, s,
                     mybir.dt.float8e4, 2,
                     1.0 / (WSCALE * XSCALE), 1.0 / WSCALE, first=(s == 0))
            for s in range(S16):
                slot(S8 + s, s * CAP, M8 + s * CAP, xT16, w1s16, w2s16, s,
                     fp16, 1, 1.0, 1.0)
    nc.compile()
    return nc


def _get_program(S8, S16, M):
    key = (S8, S16, M)
    if key not in _PROGRAM_CACHE:
        _PROGRAM_CACHE[key] = _build_program(S8, S16, M)
    return _PROGRAM_CACHE[key]


# ------------------------------------------------------------------ kernel

def _arrange_w(w):
    """[D, I] -> [io, p, ko, c] so each (slot, io) block DMAs contiguously
    into an SBUF tile laid out [partition, ko, col]."""
    return np.ascontiguousarray(
        w.reshape(KT, 128, KT, 128).transpose(2, 1, 0, 3))


def _q8(a, scale, dt):
    """Scale + saturate to TRN e4m3 range (+-240) before fp8 downcast."""
    return np.clip(a * np.float32(scale), -240.0, 240.0).astype(dt)


def kernel(x, sw1, sb1, sw2, sb2, rw1, rb1, rw2, rb2, wr, br, _trace=False):
    import ml_dtypes
    from concourse.bass_utils import run_bass_kernel_spmd

    fp8 = ml_dtypes.float8_e4m3

    x = np.asarray(x, dtype=np.float32)
    B, Sq, _ = x.shape
    T = B * Sq
    xf = np.ascontiguousarray(x.reshape(T, D))

    idx, vals = _route(xf, np.asarray(wr, np.float32), np.asarray(br, np.float32))
    pieces, S8, shared, S16, order, tok_of = _build_plan(T, idx)
    M8, M16 = S8 * CAP, S16 * CAP
    M = M8 + M16

    rw1 = np.asarray(rw1, np.float32); rw2 = np.asarray(rw2, np.float32)
    rb1 = np.asarray(rb1, np.float32); rb2 = np.asarray(rb2, np.float32)
    sw1 = np.asarray(sw1, np.float32); sw2 = np.asarray(sw2, np.float32)
    sb1 = np.asarray(sb1, np.float32); sb2 = np.asarray(sb2, np.float32)

    w1a = [_q8(_arrange_w(rw1[e]), WSCALE, fp8) for e in range(EXPERTS)]
    w2a = [_q8(_arrange_w(rw2[e]), WSCALE, fp8) for e in range(EXPERTS)]
    sw1a = _arrange_w(sw1).astype(np.float16)
    sw2a = _arrange_w(sw2).astype(np.float16)
    b1a = [np.ascontiguousarray(rb1[e].reshape(KT, 128).T) for e in range(EXPERTS)]
    b2a = [np.ascontiguousarray(rb2[e].reshape(KT, 128).T) for e in range(EXPERTS)]
    sb1a = np.ascontiguousarray(sb1.reshape(KT, 128).T)
    sb2a = np.ascontiguousarray(sb2.reshape(KT, 128).T)

    xfT = np.ascontiguousarray(xf.T)                 # [D, T] f32
    xfT8 = _q8(xfT, XSCALE, fp8)                     # routed operand
    xfT16 = xfT.astype(np.float16)                   # shared operand

    in_maps = []
    for core in range(NCORES):
        xT8c = np.zeros((D, M8), dtype=fp8)
        xT16c = np.zeros((D, M16), dtype=np.float16)
        w1c8 = np.zeros((S8, KT, 128, KT, 128), dtype=fp8)
        w2c8 = np.zeros((S8, KT, 128, KT, 128), dtype=fp8)
        w1c16 = np.zeros((S16, KT, 128, KT, 128), dtype=np.float16)
        w2c16 = np.zeros((S16, KT, 128, KT, 128), dtype=np.float16)
        b1c = np.zeros((S8 + S16, 128, KT), dtype=np.float32)
        b2c = np.zeros((S8 + S16, 128, KT), dtype=np.float32)
        for j in range(S8):
            e, a, b = pieces[core * S8 + j]
            if e >= 0:
                xT8c[:, j * CAP: j * CAP + (b - a)] = xfT8[:, tok_of[a:b]]
                w1c8[j] = w1a[e]; w2c8[j] = w2a[e]
                b1c[j] = b1a[e]; b2c[j] = b2a[e]
        for j in range(S16):
            a, b = shared[core * S16 + j]
            xT16c[:, j * CAP: j * CAP + (b - a)] = xfT16[:, a:b]
            w1c16[j] = sw1a; w2c16[j] = sw2a
            b1c[S8 + j] = sb1a; b2c[S8 + j] = sb2a
        in_maps.append({
            "xT8": xT8c.reshape(KT, 128, M8),
            "xT16": xT16c.reshape(KT, 128, M16),
            "w1s8": w1c8, "w2s8": w2c8,
            "w1s16": w1c16, "w2s16": w2c16,
            "b1s": b1c, "b2s": b2c,
        })

    nc = _get_program(S8, S16, M)
    res = run_bass_kernel_spmd(nc, in_maps, core_ids=list(range(NCORES)),
                               trace=_trace)
    kernel.last_result = res

    TK = T * TOPK
    gated = np.empty((TK, D), dtype=np.float32)   # expert-major rows
    shared_out = np.empty((T, D), dtype=np.float32)
    for core in range(NCORES):
        Y = res.results[core]["yT"].reshape(D, M)
        for j in range(S8):
            e, a, b = pieces[core * S8 + j]
            if e >= 0:
                gated[a:b] = Y[:, j * CAP: j * CAP + (b - a)].T
        for j in range(S16):
            a, b = shared[core * S16 + j]
            shared_out[a:b] = Y[:, M8 + j * CAP: M8 + j * CAP + (b - a)].T

    g = vals.ravel()[order].astype(np.float32)
    gated *= g[:, None]
    ord2 = np.argsort(tok_of, kind="stable")      # token-major, expert asc
    routed = gated[ord2].reshape(T, TOPK, D).sum(axis=1, dtype=np.float32)

    out = shared_out + routed + xf
    return out.reshape(B, Sq, D).astype(np.float32)


kernel.last_result = None


# revision 21
# speedup vs baseline: 1.1486x; 1.0707x over previous
"""MoE (63 routed experts top-7 + 1 shared expert) Trainium2 kernel.

Strategy: expert-parallel sparse dispatch. The router (softmax + top-k,
~0.3% of FLOPs) runs on host; tokens are gathered expert-major into
fixed-capacity weight slots, which are distributed across 8 NeuronCores.
Each core runs an identical (SPMD) Bass program in two phases:

  * routed phase: 8 slots x 1024 tokens in fp8e4m3 with DoubleRow
    matmuls (2 k-tiles per instruction, ~2x Tensor throughput).
    Weights are pre-scaled x64 and activations x16 on host to stay out
    of the e4m3 subnormal range; the product scale is divided back out
    via the activation unit's scale port. Gate-damping (sum g_i << 1)
    keeps the routed fp8 error small in the final output.
  * shared phase: 1 slot x 1024 tokens in fp16 (the shared expert's
    output is ungated so fp8 error there would dominate the result).

Per slot: a 1280->1280 Linear + exact GELU + 1280->1280 Linear,
feature-major (features on partitions, tokens on the free dim) so
weights need no transpose and biases ride the activation unit's
per-partition bias port. Outputs are gathered and gate-weighted back on
host in the reference's exact accumulation order.
"""

import math
import sys

sys.path.insert(0, "/opt/trn_rl_repo")

import numpy as np

D = 1280          # model dim
I = 1280          # expert inter dim
EXPERTS = 63      # routed experts
TOPK = 7          # routed top-k
CAP = 1024        # tokens per weight slot
CHUNK = 512       # tokens per matmul (PSUM bank limit)
KT = D // 128     # 10 contraction tiles
NCORES = 8

# fp8 scaling: weights x64 and activations x16 keep values out of the e4m3
# subnormal range (|v| < 2^-6); the product scale 1/1024 (layer 1) and 1/64
# (layer 2) is folded into the activation's scale port.
WSCALE = 64.0
XSCALE = 16.0

_PROGRAM_CACHE = {}


# ----------------------------------------------------------------- router

def _route(x2d, wr, br):
    """f32 softmax + top-k, matching jax.nn.softmax / jax.lax.top_k."""
    logits = (x2d @ wr + br).astype(np.float32)
    logits -= logits.max(-1, keepdims=True)
    np.exp(logits, out=logits)
    aff = logits / logits.sum(-1, keepdims=True)
    idx = np.argsort(-aff, axis=-1, kind="stable")[:, :TOPK]
    vals = np.take_along_axis(aff, idx, axis=-1)
    return idx.astype(np.int32), vals.astype(np.float32)


def _build_plan(T, idx):
    """Routed (token, expert) pairs packed expert-major into CAP-token
    pieces -> 8 cores x S8 fp8 slots (dummy-padded); shared tokens ->
    8 cores x S16 fp16 slots."""
    flat = idx.ravel()
    order = np.argsort(flat, kind="stable")          # expert-major slot order
    tok_of = (order // TOPK).astype(np.int64)
    counts = np.bincount(flat, minlength=EXPERTS)
    offs = np.concatenate([[0], np.cumsum(counts)])

    pieces = []  # (expert, a, b)  [a:b) into the expert-major order
    for e in range(EXPERTS):
        a, b = int(offs[e]), int(offs[e + 1])
        while a < b:
            n = min(CAP, b - a)
            pieces.append((e, a, a + n))
            a += n

    S8 = max(1, math.ceil(len(pieces) / NCORES))
    pieces += [(-1, 0, 0)] * (NCORES * S8 - len(pieces))   # dummy cells

    # shared tokens: contiguous ranges, T/NCORES per core in S16 slots
    per_core = math.ceil(T / NCORES)
    S16 = max(1, math.ceil(per_core / CAP))
    shared = []  # (a, b) token range per (core, slot)
    for c in range(NCORES):
        t0, t1 = min(c * per_core, T), min((c + 1) * per_core, T)
        for j in range(S16):
            a = min(t0 + j * CAP, t1)
            shared.append((a, min(a + CAP, t1)))
    return pieces, S8, shared, S16, order, tok_of


# ----------------------------------------------------------- device program

def _build_program(S8, S16, M):
    import concourse.mybir as mybir
    import concourse.tile as tile
    from concourse import bacc

    f32 = mybir.dt.float32
    fp8 = mybir.dt.float8e4
    fp16 = mybir.dt.float16

    M8 = S8 * CAP
    M16 = S16 * CAP
    assert M == M8 + M16

    nc = bacc.Bacc("TRN2", target_bir_lowering=False, debug=False,
                   enable_asserts=False, num_devices=NCORES)
    xT8 = nc.dram_tensor("xT8", [KT, 128, M8], fp8, kind="ExternalInput").ap()
    xT16 = nc.dram_tensor("xT16", [KT, 128, M16], fp16, kind="ExternalInput").ap()
    w1s8 = nc.dram_tensor("w1s8", [S8, KT, 128, KT, 128], fp8, kind="ExternalInput").ap()
    w2s8 = nc.dram_tensor("w2s8", [S8, KT, 128, KT, 128], fp8, kind="ExternalInput").ap()
    w1s16 = nc.dram_tensor("w1s16", [S16, KT, 128, KT, 128], fp16, kind="ExternalInput").ap()
    w2s16 = nc.dram_tensor("w2s16", [S16, KT, 128, KT, 128], fp16, kind="ExternalInput").ap()
    b1s = nc.dram_tensor("b1s", [S8 + S16, 128, KT], f32, kind="ExternalInput").ap()
    b2s = nc.dram_tensor("b2s", [S8 + S16, 128, KT], f32, kind="ExternalInput").ap()
    yT = nc.dram_tensor("yT", [KT, 128, M], f32, kind="ExternalOutput").ap()

    CPS = CAP // CHUNK  # chunks per slot
    Gelu = mybir.ActivationFunctionType.Gelu
    DR = mybir.MatmulPerfMode.DoubleRow

    with tile.TileContext(nc) as tc:
        with (
            tc.tile_pool(name="xa", bufs=3) as xa,
            tc.tile_pool(name="w1p", bufs=6) as w1p,
            tc.tile_pool(name="w2p", bufs=6) as w2p,
            tc.tile_pool(name="hp", bufs=3) as hp,
            tc.tile_pool(name="yo", bufs=6) as yo,
            tc.tile_pool(name="bp", bufs=8) as bp,
            tc.tile_pool(name="ps", bufs=8, space="PSUM") as ps,
        ):
            def slot(s, xcol0, ycol0, xsrc, w1src, w2src, widx, in_dt, kstep,
                     s1, s2, first=False):
                pm = DR if kstep == 2 else None
                b1t = bp.tile([128, KT], f32, tag="b1", name="b1t")
                nc.sync.dma_start(out=b1t[:, :], in_=b1s[s])
                b2t = bp.tile([128, KT], f32, tag="b2", name="b2t")
                nc.sync.dma_start(out=b2t[:, :], in_=b2s[s])

                xc = []
                for c in range(CPS):
                    xt = xa.tile([128, KT, CHUNK], in_dt, tag=f"x{kstep}",
                                 name="xt")
                    for k in range(KT):
                        # SWDGE: keeps HWDGE free for the slot's weight
                        # loads.  First slot: split across the still-idle
                        # HWDGE queues to shorten kernel startup.
                        eng = nc.gpsimd if not first or k % 2 else nc.scalar
                        eng.dma_start(
                            out=xt[:, k, :],
                            in_=xsrc[k, :, xcol0 + c * CHUNK:
                                     xcol0 + (c + 1) * CHUNK])
                    xc.append(xt)

                hc = [hp.tile([128, KT, CHUNK], in_dt, tag=f"h{kstep}",
                              name=f"h{c}") for c in range(CPS)]

                # layer 1: h = gelu((x @ w1) * s1 + b1), feature-major
                for io in range(KT):
                    w1t = w1p.tile([128, KT, 128], in_dt, tag=f"w1{kstep}",
                                   name="w1t")
                    nc.sync.dma_start(out=w1t[:, :, :], in_=w1src[widx, io])
                    pts = [ps.tile([128, CHUNK], f32, tag="ps", name="pt")
                           for _ in range(CPS)]
                    # k outer, c inner: one weight (pair) load covers CPS
                    # matmuls so LDWEIGHTS hides under the streaming
                    for k in range(0, KT, kstep):
                        for c in range(CPS):
                            nc.tensor.matmul(pts[c][:, :],
                                             w1t[:, k:k + kstep, :],
                                             xc[c][:, k:k + kstep, :],
                                             start=(k == 0),
                                             stop=(k + kstep >= KT),
                                             perf_mode=pm)
                    for c in range(CPS):
                        nc.scalar.activation(hc[c][:, io, :], pts[c][:, :],
                                             Gelu, bias=b1t[:, io:io + 1],
                                             scale=s1)

                # layer 2: y = (h @ w2) * s2 + b2
                for io in range(KT):
                    w2t = w2p.tile([128, KT, 128], in_dt, tag=f"w2{kstep}",
                                   name="w2t")
                    nc.sync.dma_start(out=w2t[:, :, :], in_=w2src[widx, io])
                    pts = [ps.tile([128, CHUNK], f32, tag="ps", name="pt")
                           for _ in range(CPS)]
                    for k in range(0, KT, kstep):
                        for c in range(CPS):
                            nc.tensor.matmul(pts[c][:, :],
                                             w2t[:, k:k + kstep, :],
                                             hc[c][:, k:k + kstep, :],
                                             start=(k == 0),
                                             stop=(k + kstep >= KT),
                                             perf_mode=pm)
                    for c in range(CPS):
                        yt = yo.tile([128, CHUNK], f32, tag="y", name="yt")
                        # drain layer-2 PSUM on the (otherwise idle) Vector
                        # engine: y = psum * s2 + b2.  DVE at ~267ns/tile
                        # beats PE production (~1080ns/tile), so slot tails
                        # don't serialize behind the Scalar engine.
                        nc.vector.tensor_scalar(
                            yt[:, :], pts[c][:, :], s2, b2t[:, io:io + 1],
                            mybir.AluOpType.mult, mybir.AluOpType.add)
                        # y-stores ride the Scalar HWDGE queue so the Sync
                        # queue can issue the NEXT slot's weight loads
                        # without queuing behind 20 gated stores
                        nc.scalar.dma_start(
                            out=yT[io, :, ycol0 + c * CHUNK:
                                   ycol0 + (c + 1) * CHUNK],
                            in_=yt[:, :])

            for s in range(S8):
                slot(s, s * CAP, s * CAP, xT8, w1s8, w2s8, s,
                     mybir.dt.float8e4, 2,
                     1.0 / (WSCALE * XSCALE), 1.0 / WSCALE, first=(s == 0))
            for s in range(S16):
                slot(S8 + s, s * CAP, M8 + s * CAP, xT16, w1s16, w2s16, s,
                     fp16, 1, 1.0, 1.0)
    nc.compile()
    return nc


def _get_program(S8, S16, M):
    key = (S8, S16, M)
    if key not in _PROGRAM_CACHE:
        _PROGRAM_CACHE[key] = _build_program(S8, S16, M)
    return _PROGRAM_CACHE[key]


# ------------------------------------------------------------------ kernel

def _arrange_w(w):
    """[D, I] -> [io, p, ko, c] so each (slot, io) block DMAs contiguously
    into an SBUF tile laid out [partition, ko, col]."""
    return np.ascontiguousarray(
        w.reshape(KT, 128, KT, 128).transpose(2, 1, 0, 3))


def _q8(a, scale, dt):
    """Scale + saturate to TRN e4m3 range (+-240) before fp8 downcast."""
    return np.clip(a * np.float32(scale), -240.0, 240.0).astype(dt)


def kernel(x, sw1, sb1, sw2, sb2, rw1, rb1, rw2, rb2, wr, br, _trace=False):
    import ml_dtypes
    from concourse.bass_utils import run_bass_kernel_spmd

    fp8 = ml_dtypes.float8_e4m3

    x = np.asarray(x, dtype=np.float32)
    B, Sq, _ = x.shape
    T = B * Sq
    xf = np.ascontiguousarray(x.reshape(T, D))

    idx, vals = _route(xf, np.asarray(wr, np.float32), np.asarray(br, np.float32))
    pieces, S8, shared, S16, order, tok_of = _build_plan(T, idx)
    M8, M16 = S8 * CAP, S16 * CAP
    M = M8 + M16

    rw1 = np.asarray(rw1, np.float32); rw2 = np.asarray(rw2, np.float32)
    rb1 = np.asarray(rb1, np.float32); rb2 = np.asarray(rb2, np.float32)
    sw1 = np.asarray(sw1, np.float32); sw2 = np.asarray(sw2, np.float32)
    sb1 = np.asarray(sb1, np.float32); sb2 = np.asarray(sb2, np.float32)

    w1a = [_q8(_arrange_w(rw1[e]), WSCALE, fp8) for e in range(EXPERTS)]
    w2a = [_q8(_arrange_w(rw2[e]), WSCALE, fp8) for e in range(EXPERTS)]
    sw1a = _arrange_w(sw1).astype(np.float16)
    sw2a = _arrange_w(sw2).astype(np.float16)
    b1a = [np.ascontiguousarray(rb1[e].reshape(KT, 128).T) for e in range(EXPERTS)]
    b2a = [np.ascontiguousarray(rb2[e].reshape(KT, 128).T) for e in range(EXPERTS)]
    sb1a = np.ascontiguousarray(sb1.reshape(KT, 128).T)
    sb2a = np.ascontiguousarray(sb2.reshape(KT, 128).T)

    xfT = np.ascontiguousarray(xf.T)                 # [D, T] f32
    xfT8 = _q8(xfT, XSCALE, fp8)                     # routed operand
    xfT16 = xfT.astype(np.float16)                   # shared operand

    in_maps = []
    for core in range(NCORES):
        xT8c = np.zeros((D, M8), dtype=fp8)
        xT16c = np.zeros((D, M16), dtype=np.float16)
        w1c8 = np.zeros((S8, KT, 128, KT, 128), dtype=fp8)
        w2c8 = np.zeros((S8, KT, 128, KT, 128), dtype=fp8)
        w1c16 = np.zeros((S16, KT, 128, KT, 128), dtype=np.float16)
        w2c16 = np.zeros((S16, KT, 128, KT, 128), dtype=np.float16)
        b1c = np.zeros((S8 + S16, 128, KT), dtype=np.float32)
        b2c = np.zeros((S8 + S16, 128, KT), dtype=np.float32)
        for j in range(S8):
            e, a, b = pieces[core * S8 + j]
            if e >= 0:
                xT8c[:, j * CAP: j * CAP + (b - a)] = xfT8[:, tok_of[a:b]]
                w1c8[j] = w1a[e]; w2c8[j] = w2a[e]
                b1c[j] = b1a[e]; b2c[j] = b2a[e]
        for j in range(S16):
            a, b = shared[core * S16 + j]
            xT16c[:, j * CAP: j * CAP + (b - a)] = xfT16[:, a:b]
            w1c16[j] = sw1a; w2c16[j] = sw2a
            b1c[S8 + j] = sb1a; b2c[S8 + j] = sb2a
        in_maps.append({
            "xT8": xT8c.reshape(KT, 128, M8),
            "xT16": xT16c.reshape(KT, 128, M16),
            "w1s8": w1c8, "w2s8": w2c8,
            "w1s16": w1c16, "w2s16": w2c16,
            "b1s": b1c, "b2s": b2c,
        })

    nc = _get_program(S8, S16, M)
    res = run_bass_kernel_spmd(nc, in_maps, core_ids=list(range(NCORES)),
                               trace=_trace)
    kernel.last_result = res

    TK = T * TOPK
    gated = np.empty((TK, D), dtype=np.float32)   # expert-major rows
    shared_out = np.empty((T, D), dtype=np.float32)
    for core in range(NCORES):
        Y = res.results[core]["yT"].reshape(D, M)
        for j in range(S8):
            e, a, b = pieces[core * S8 + j]
            if e >= 0:
                gated[a:b] = Y[:, j * CAP: j * CAP + (b - a)].T
        for j in range(S16):
            a, b = shared[core * S16 + j]
            shared_out[a:b] = Y[:, M8 + j * CAP: M8 + j * CAP + (b - a)].T

    g = vals.ravel()[order].astype(np.float32)
    gated *= g[:, None]
    ord2 = np.argsort(tok_of, kind="stable")      # token-major, expert asc
    routed = gated[ord2].reshape(T, TOPK, D).sum(axis=1, dtype=np.float32)

    out = shared_out + routed + xf
    return out.reshape(B, Sq, D).astype(np.float32)


kernel.last_result = None
